# revision 9
# baseline (speedup 1.0000x reference)
"""Trainium2 Bass kernel for nn_DAM2_68934225101109 (fused DAM block).

Self-contained: kernel(**inputs) takes the full [8,256,128,128] inputs,
shards one image per NeuronCore (8 cores), runs a fused Bass/Tile kernel,
and gathers the full [8,256,128,128] float32 output.

v1 restructure vs baseline:
- x_c kept resident in SBUF; score convs algebraically folded through
  K = b1_w^T @ b2_w so the raw x_c feeds the per-row score matmuls
  directly (channel gate cg applied at conv evictions via activation
  scale).  Eliminates the b1(x_c)/b2(x_c) convs and their DRAM spills.
- PA gate computed as a single 4-matrix PSUM accumulation with
  Wca@W2 / Wcb@W2 folded host-side (no g19/g2 intermediates).
- Two loops: loop1 produces bp (spilled) + the validity-mask input;
  morphology; loop2 recomputes scores (M_c_to_p), attends, and fuses.
"""
from contextlib import ExitStack

import numpy as np
import ml_dtypes

import bass_rust
import concourse.bass as bass
import concourse.mybir as mybir
import concourse.tile as tile
from concourse.masks import make_identity
from concourse.bass_utils import run_bass_kernel_spmd

_ctr = [0]


def split_multi_waits(nc):
    n_split = 0
    for f in nc.m.functions:
        for b in f.blocks:
            out = []
            changed = False
            for inst in b.instructions:
                si = inst.sync_info
                waits = list(si.on_wait) if si and si.on_wait else []
                if len(waits) > 1:
                    changed = True
                    n_split += 1
                    for w in waits[:-1]:
                        _ctr[0] += 1
                        nop = mybir.InstNoOp(
                            name=f"I-wsplit-{_ctr[0]}", ins=[], outs=[])
                        nop.engine = inst.engine
                        nop.sync_info = bass_rust.SyncInfo(
                            on_wait=[w], on_update=[])
                        nc.register_instruction(nop)
                        out.append(nop)
                    si.on_wait = waits[-1:]
                out.append(inst)
            if changed:
                b.instructions = out
    return n_split


class SplitDrainTileContext(tile.TileContext):
    """TileContext that splits multi-wait instructions on exit."""

    def __exit__(self, exc_type, exc_val, exc_tb):
        r = super().__exit__(exc_type, exc_val, exc_tb)
        if exc_type is None:
            split_multi_waits(self.nc)
        return r


BF = ml_dtypes.bfloat16
C, H, W = 256, 128, 128
HW = H * W


def _blocks(Wm):
    """W [out, in] -> lhsT blocks [ci(128), gi, go, co(128)] from W.T."""
    Wt = np.ascontiguousarray(Wm.T)  # [in, out]
    return Wt.reshape(2, 128, 2, 128).transpose(1, 0, 2, 3)


def _bias2(v):
    """[256] -> [co 128, go 2]"""
    return np.ascontiguousarray(v.reshape(2, 128).T)


def prep_shared(inp):
    """Weights/biases shared by all cores. Returns dict name->np array."""
    f32 = np.float32
    pa_w1 = inp["pa_w1"].astype(f32)
    pa_w2 = inp["pa_w2"].astype(f32)
    pa_wc = inp["pa_wc"].astype(f32)
    Wca = pa_wc[:, :256]
    Wcb = pa_wc[:, 256:]
    fus_w = inp["fus_w"].astype(f32)
    fus_w1 = fus_w[:, :256]
    fus_w2 = fus_w[:, 256:512]
    b1w = inp["b1_w"].astype(f32)
    b2w = inp["b2_w"].astype(f32)
    Kmat = b1w.T @ b2w
    Gm = fus_w1 @ inp["b3_w"].astype(f32)
    d = {
        "w1a": _blocks(pa_w1 / 9.0).astype(BF),
        "w1b": _blocks(pa_w1).astype(BF),
        "wca9": _blocks(Wca / 9.0).astype(BF),
        "wcaw2": _blocks(Wca @ pa_w2).astype(BF),
        "wcb": _blocks(Wcb).astype(BF),
        "wcbw2": _blocks(Wcb @ pa_w2).astype(BF),
        "wp": _blocks(Kmat).astype(BF),
        "wr": _blocks(np.ascontiguousarray(Kmat.T)).astype(BF),
        "f2t": _blocks(fus_w2).astype(BF),
        "gt": _blocks(Gm).astype(BF),
        "bi_b1": _bias2(inp["pa_b1"].astype(f32)),
        "bi_bc2": _bias2(inp["pa_bc"].astype(f32)
                         + (Wca + Wcb) @ inp["pa_b2"].astype(f32)),
        "bi_u": _bias2(b2w.T @ inp["b1_b"].astype(f32)),
        "bi_cb": _bias2(inp["fus_b"].astype(f32)
                        + fus_w1 @ inp["b3_b"].astype(f32)),
        "fwvrow": np.ascontiguousarray(
            fus_w[:, 512].reshape(2, 128)[None]).astype(BF),
        "bi_cab2": _bias2(inp["ca_ab2"].astype(f32)
                          + inp["ca_mb2"].astype(f32)),
        "aw1t": np.ascontiguousarray(
            (inp["ca_aw1"].astype(f32) / HW).T.reshape(2, 128, 16)
            .transpose(1, 0, 2)),
        "mw1t": np.ascontiguousarray(
            inp["ca_mw1"].astype(f32).T.reshape(2, 128, 16)
            .transpose(1, 0, 2)),
        "aw2t": np.ascontiguousarray(
            inp["ca_aw2"].astype(f32).T.reshape(16, 2, 128)),
        "mw2t": np.ascontiguousarray(
            inp["ca_mw2"].astype(f32).T.reshape(16, 2, 128)),
        "cab1a": inp["ca_ab1"].astype(f32)[:, None],
        "cab1m": inp["ca_mb1"].astype(f32)[:, None],
    }
    for k, v in d.items():
        d[k] = np.ascontiguousarray(v)
    return d


def prep_image(x):
    """[256,128,128] f32 -> [128, 2, HW] bf16"""
    return np.ascontiguousarray(
        x.reshape(2, 128, HW).transpose(1, 0, 2)).astype(BF)


def post_image(y):
    """[128, 2, HW] f32 -> [256,128,128] f32"""
    return np.ascontiguousarray(
        y.astype(np.float32).transpose(1, 0, 2)).reshape(256, 128, 128)


F32 = mybir.dt.float32
BF16 = mybir.dt.bfloat16
AF = mybir.ActivationFunctionType
ALU = mybir.AluOpType
AX = mybir.AxisListType

SR = 16              # strip rows
PX = SR * W          # 2048 strip pixels
NS = H // SR         # 8 strips


def _disk_row_widths(r):
    out = {}
    for dy in range(-r, r + 1):
        dx = int(np.floor(np.sqrt(r * r - dy * dy)))
        out[dy] = 2 * dx + 1
    return out


def band_matrix(in_rows, out_rows, in_off, out_off, dys):
    """T[q, p] = 1 if (out_off + p) - (in_off + q) in dys. lhsT layout."""
    T = np.zeros((in_rows, out_rows), np.float32)
    for q in range(in_rows):
        for p in range(out_rows):
            if (out_off + p) - (in_off + q) in dys:
                T[q, p] = 1.0
    return T


def build(nc):
    # ---- DRAM I/O ----
    xp = nc.dram_tensor("xp", [128, 2, HW], BF16, kind="ExternalInput")
    xc = nc.dram_tensor("xc", [128, 2, HW], BF16, kind="ExternalInput")
    wnames = ["w1a", "w1b", "wca9", "wcaw2", "wcb", "wcbw2", "wp", "wr",
              "f2t", "gt"]
    wd = {n: nc.dram_tensor(n, [128, 2, 2, 128], BF16, kind="ExternalInput")
          for n in wnames}
    bnames = ["bi_b1", "bi_bc2", "bi_u", "bi_cb", "bi_cab2"]
    bd = {n: nc.dram_tensor(n, [128, 2], F32, kind="ExternalInput")
          for n in bnames}
    aw1t = nc.dram_tensor("aw1t", [128, 2, 16], F32, kind="ExternalInput")
    mw1t = nc.dram_tensor("mw1t", [128, 2, 16], F32, kind="ExternalInput")
    aw2t = nc.dram_tensor("aw2t", [16, 2, 128], F32, kind="ExternalInput")
    mw2t = nc.dram_tensor("mw2t", [16, 2, 128], F32, kind="ExternalInput")
    cab1a = nc.dram_tensor("cab1a", [16, 1], F32, kind="ExternalInput")
    cab1m = nc.dram_tensor("cab1m", [16, 1], F32, kind="ExternalInput")
    fwvrow = nc.dram_tensor("fwvrow", [1, 2, 128], BF16, kind="ExternalInput")

    y = nc.dram_tensor("y", [128, 2, HW], F32, kind="ExternalOutput")

    # DRAM scratch
    bpd = nc.dram_tensor("bpd", [128, 2, HW], BF16, kind="Internal")
    vfd = nc.dram_tensor("vfd", [1, HW], BF16, kind="Internal")

    # morphology band matrices
    d1w, d2w, d3w = _disk_row_widths(1), _disk_row_widths(2), _disk_row_widths(3)

    def cls_groups(wmap):
        g = {}
        for dy, wdt in wmap.items():
            g.setdefault(wdt, []).append(dy)
        return g

    bands = {}

    for nm, wmap in [("d1", d1w), ("d2", d2w)]:
        for wdt, dys in cls_groups(wmap).items():
            bands[f"{nm}_w{wdt}"] = band_matrix(128, 128, 0, 0, dys)
    for wdt, dys in cls_groups(d3w).items():
        bands[f"d3a_w{wdt}"] = band_matrix(128, 67, 0, -3, dys)
        bands[f"d3b_w{wdt}"] = band_matrix(128, 67, 0, 64, dys)
    for wdt, dys in cls_groups(d3w).items():
        bands[f"e3a_w{wdt}"] = band_matrix(67, 128, -3, 0, dys)
        bands[f"e3b_w{wdt}"] = band_matrix(67, 128, 64, 0, dys)
    band_dram = {n: nc.inline_tensor(a.astype(ml_dtypes.bfloat16),
                                     name=f"bm_{n}")
                 for n, a in bands.items()}

    with SplitDrainTileContext(nc, pool_alloc_mode="queue") as tc, \
            ExitStack() as top:
        # ---------- persistent pool ----------
        wpool = top.enter_context(tc.tile_pool(name="wts", bufs=1))
        wt = {n: wpool.tile([128, 2, 2, 128], BF16, tag=n, name=n)
              for n in wnames}
        for n in wnames:
            nc.sync.dma_start(wt[n][:], wd[n][:])
        bt = {n: wpool.tile([128, 2], F32, tag=n, name=n) for n in bnames}
        for n in bnames:
            nc.sync.dma_start(bt[n][:], bd[n][:])
        t_aw1 = wpool.tile([128, 2, 16], F32, tag="aw1")
        t_mw1 = wpool.tile([128, 2, 16], F32, tag="mw1")
        t_aw2 = wpool.tile([16, 2, 128], F32, tag="aw2")
        t_mw2 = wpool.tile([16, 2, 128], F32, tag="mw2")
        t_cab1a = wpool.tile([16, 1], F32, tag="cab1a")
        t_cab1m = wpool.tile([16, 1], F32, tag="cab1m")
        for t, d in [(t_aw1, aw1t), (t_mw1, mw1t), (t_aw2, aw2t),
                     (t_mw2, mw2t), (t_cab1a, cab1a), (t_cab1m, cab1m)]:
            nc.sync.dma_start(t[:], d[:])
        identb = wpool.tile([128, 128], BF16, tag="identb")
        make_identity(nc, identb[:])
        t_fwvrow = wpool.tile([1, 2, 128], BF16, tag="fwvrow")
        nc.sync.dma_start(t_fwvrow[:], fwvrow[:])
        # stats accumulators + gate
        sums = wpool.tile([128, 2, NS], F32, tag="sums")
        maxs = wpool.tile([128, 2, NS], F32, tag="maxs")
        cg = wpool.tile([128, 2], F32, tag="cg")
        cgu = wpool.tile([128, 2], F32, tag="cgu")
        # resident x_c image [128, 2, HW] bf16 (64KB/partition)
        XC = wpool.tile([128, 2, HW], BF16, tag="XC")
        # mask rows (inverted) for morphology input
        m0 = wpool.tile([128, W], BF16, tag="m0")

        # ================= pre-pass: load XC + stats =================
        with tc.tile_pool(name="pS", bufs=2) as pS:
            for s in range(NS):
                px0 = s * PX
                nc.sync.dma_start(XC[:, :, px0:px0 + PX],
                                  xc[:, :, px0:px0 + PX])
                for g in range(2):
                    dumb = pS.tile([128, PX], BF16, tag="dumb")
                    nc.scalar.activation(dumb[:], XC[:, g, px0:px0 + PX],
                                         AF.Copy,
                                         accum_out=sums[:, g, s:s + 1])
                nc.vector.tensor_reduce(maxs[:, :, s:s + 1],
                                        XC[:, :, px0:px0 + PX],
                                        AX.X, ALU.max)

        # ================= gate (CA MLP) =================
        with tc.tile_pool(name="pG", bufs=1) as pG, \
                tc.tile_pool(name="psG", bufs=1, space="PSUM") as psG:
            avec = pG.tile([128, 2], F32, tag="avec")
            nc.vector.tensor_reduce(avec[:], sums[:], AX.X, ALU.add)
            mvec = pG.tile([128, 2], F32, tag="mvec")
            nc.vector.tensor_reduce(mvec[:], maxs[:], AX.X, ALU.max)
            ta_ = pG.tile([16, 1], F32, tag="ta")
            tm_ = pG.tile([16, 1], F32, tag="tm")
            for (w1, vec, b1t_, dst) in [(t_aw1, avec, t_cab1a, ta_),
                                         (t_mw1, mvec, t_cab1m, tm_)]:
                pp = psG.tile([16, 1], F32, tag="pmlp1")
                for g in range(2):
                    nc.tensor.matmul(pp[:], w1[:, g, :], vec[:, g:g + 1],
                                     start=(g == 0), stop=(g == 1))
                nc.scalar.activation(dst[:], pp[:], AF.Relu, bias=b1t_[:])
            for go in range(2):
                pp = psG.tile([128, 1], F32, tag="pmlp2")
                nc.tensor.matmul(pp[:], t_aw2[:, go, :], ta_[:],
                                 start=True, stop=False)
                nc.tensor.matmul(pp[:], t_mw2[:, go, :], tm_[:],
                                 start=False, stop=True)
                nc.scalar.activation(cg[:, go:go + 1], pp[:], AF.Sigmoid,
                                     bias=bt["bi_cab2"][:, go:go + 1])
            nc.vector.tensor_tensor(cgu[:], cg[:], bt["bi_u"][:], ALU.mult)

        # ================= loop1 =================
        with tc.tile_pool(name="pA", bufs=2) as pA, \
                tc.tile_pool(name="pA1", bufs=1) as pA1, \
                tc.tile_pool(name="psA", bufs=4, space="PSUM") as psA, \
                tc.tile_pool(name="psAs", bufs=2, space="PSUM") as psAs, \
                tc.tile_pool(name="psAc", bufs=1, space="PSUM") as psAc:

            def conv(dst, mats, evict, nkb=PX // 512):
                """dst[:, go, k*512:(k+1)*512] = evict(sum over (w, src, gi))

                mats: list of (wtile, src_tile_or_view).  src indexed
                [:, gi, slice]."""
                for go in range(2):
                    for kb in range(nkb):
                        sl = slice(kb * 512, (kb + 1) * 512)
                        pp = psA.tile([128, 512], F32, tag="pconv")
                        nmm = 2 * len(mats)
                        i = 0
                        for (wtile, src) in mats:
                            for gi in range(2):
                                nc.tensor.matmul(
                                    pp[:], wtile[:, gi, go, :], src[:, gi, sl],
                                    start=(i == 0), stop=(i == nmm - 1))
                                i += 1
                        evict(dst, pp, go, sl)

            def ev_scalar(func, bias_tile=None, scale_tile=None):
                def f(dst, pp, go, sl):
                    kw = {}
                    if bias_tile is not None:
                        kw["bias"] = bias_tile[:, go:go + 1]
                    if scale_tile is not None:
                        kw["scale"] = scale_tile[:, go:go + 1]
                    nc.scalar.activation(dst[:, go, sl], pp[:], func, **kw)
                return f

            for s in range(NS):
                px0 = s * PX
                # ---- x_p halo strip [128, 2, 18*128] ----
                xph = pA.tile([128, 2, 18 * W], BF16, tag="xph")
                if s == 0:
                    nc.vector.memset(xph[:, :, 0:W], 0.0)
                    nc.sync.dma_start(xph[:, :, W:], xp[:, :, 0:17 * W])
                elif s == NS - 1:
                    nc.sync.dma_start(xph[:, :, :17 * W], xp[:, :, px0 - W:])
                    nc.vector.memset(xph[:, :, 17 * W:], 0.0)
                else:
                    nc.sync.dma_start(xph[:], xp[:, :, px0 - W:px0 + 17 * W])
                x4 = xph[:].rearrange("p g (r w) -> p g r w", w=W)

                # ---- horizontal 3-window sum/max (18 rows) ----
                hs = pA1.tile([128, 2, 18, W], BF16, tag="hs")
                nc.vector.tensor_tensor(hs[:, :, :, 1:127], x4[:, :, :, 0:126],
                                        x4[:, :, :, 1:127], ALU.add)
                nc.vector.tensor_tensor(hs[:, :, :, 1:127], hs[:, :, :, 1:127],
                                        x4[:, :, :, 2:128], ALU.add)
                nc.vector.tensor_tensor(hs[:, :, :, 0:1], x4[:, :, :, 0:1],
                                        x4[:, :, :, 1:2], ALU.add)
                nc.vector.tensor_tensor(hs[:, :, :, 127:128],
                                        x4[:, :, :, 126:127],
                                        x4[:, :, :, 127:128], ALU.add)
                hm = pA1.tile([128, 2, 18, W], BF16, tag="hm")
                nc.vector.tensor_tensor(hm[:, :, :, 1:127], x4[:, :, :, 0:126],
                                        x4[:, :, :, 1:127], ALU.max)
                nc.vector.tensor_tensor(hm[:, :, :, 1:127], hm[:, :, :, 1:127],
                                        x4[:, :, :, 2:128], ALU.max)
                nc.vector.tensor_tensor(hm[:, :, :, 0:1], x4[:, :, :, 0:1],
                                        x4[:, :, :, 1:2], ALU.max)
                nc.vector.tensor_tensor(hm[:, :, :, 127:128],
                                        x4[:, :, :, 126:127],
                                        x4[:, :, :, 127:128], ALU.max)

                # ---- vertical 3-window -> ys (=9*avg3), ym (=max3) ----
                ys = pA.tile([128, 2, PX], BF16, tag="ys")
                y4v = ys[:].rearrange("p g (r w) -> p g r w", w=W)
                nc.vector.tensor_tensor(y4v[:], hs[:, :, 0:16, :],
                                        hs[:, :, 1:17, :], ALU.add)
                nc.vector.tensor_tensor(y4v[:], y4v[:], hs[:, :, 2:18, :],
                                        ALU.add)
                ym = pA.tile([128, 2, PX], BF16, tag="ym")
                m4v = ym[:].rearrange("p g (r w) -> p g r w", w=W)
                if s == 0:
                    nc.vector.tensor_tensor(m4v[:, :, 1:16, :],
                                            hm[:, :, 1:16, :],
                                            hm[:, :, 2:17, :], ALU.max)
                    nc.vector.tensor_tensor(m4v[:, :, 1:16, :],
                                            m4v[:, :, 1:16, :],
                                            hm[:, :, 3:18, :], ALU.max)
                    nc.vector.tensor_tensor(m4v[:, :, 0:1, :],
                                            hm[:, :, 1:2, :],
                                            hm[:, :, 2:3, :], ALU.max)
                elif s == NS - 1:
                    nc.vector.tensor_tensor(m4v[:, :, 0:15, :],
                                            hm[:, :, 0:15, :],
                                            hm[:, :, 1:16, :], ALU.max)
                    nc.vector.tensor_tensor(m4v[:, :, 0:15, :],
                                            m4v[:, :, 0:15, :],
                                            hm[:, :, 2:17, :], ALU.max)
                    nc.vector.tensor_tensor(m4v[:, :, 15:16, :],
                                            hm[:, :, 15:16, :],
                                            hm[:, :, 16:17, :], ALU.max)
                else:
                    nc.vector.tensor_tensor(m4v[:], hm[:, :, 0:16, :],
                                            hm[:, :, 1:17, :], ALU.max)
                    nc.vector.tensor_tensor(m4v[:], m4v[:], hm[:, :, 2:18, :],
                                            ALU.max)

                # ---- PA chain ----
                t1 = pA1.tile([128, 2, PX], BF16, tag="t1")
                conv(t1, [(wt["w1a"], ys)], ev_scalar(AF.Relu, bt["bi_b1"]))
                t2 = pA1.tile([128, 2, PX], BF16, tag="t2")
                conv(t2, [(wt["w1b"], ym)], ev_scalar(AF.Relu, bt["bi_b1"]))
                pg = pA1.tile([128, 2, PX], BF16, tag="pg")
                conv(pg, [(wt["wca9"], ys), (wt["wcaw2"], t1),
                          (wt["wcb"], ym), (wt["wcbw2"], t2)],
                     ev_scalar(AF.Sigmoid, bt["bi_bc2"]))

                bp = pA.tile([128, 2, PX], BF16, tag="bp")
                xpsl = xph[:, :, W:W + PX]
                nc.vector.tensor_tensor(bp[:], pg[:], xpsl, ALU.mult)
                nc.sync.dma_start(bpd[:, :, px0:px0 + PX], bp[:])

                # ---- P = cg*(K bp) ;  E_B = XC^T P ; colsum mask ----
                Pt = pA1.tile([128, 2, PX], BF16, tag="Pt")
                conv(Pt, [(wt["wp"], bp)],
                     ev_scalar(AF.Identity, None, cg))
                for q in range(4):
                    pl = psAs.tile([128, 4, 128], F32, tag="plB")
                    for r in range(4):
                        oo = px0 + (q * 4 + r) * W
                        for gi in range(2):
                            nc.tensor.matmul(pl[:, r, :],
                                             XC[:, gi, oo:oo + W],
                                             Pt[:, gi, q * 512 + r * W:
                                                q * 512 + r * W + W],
                                             start=(gi == 0), stop=(gi == 1))
                    E = pA.tile([128, 4, 128], BF16, tag="EB")
                    nc.scalar.activation(E[:], pl[:], AF.Exp)
                    rs = pA.tile([128, 4], F32, tag="rsB")
                    nc.vector.tensor_reduce(rs[:], E[:], AX.X, ALU.add)
                    rr = pA.tile([128, 4], BF16, tag="rrB")
                    with nc.allow_low_precision(reason="colsum mask rcp"):
                        nc.vector.reciprocal(rr[:], rs[:])
                    pc = psAc.tile([1, 4, 128], F32, tag="pcB")
                    for r in range(4):
                        nc.tensor.matmul(pc[0:1, r, :], rr[:, r:r + 1],
                                         E[:, r, :], start=True, stop=True)
                    vws = pA.tile([1, 512], BF16, tag="vws")
                    nc.vector.tensor_single_scalar(
                        vws[:], pc[0:1, :, :], 0.1, ALU.is_le)
                    nc.sync.dma_start(
                        m0[s * SR + q * 4:s * SR + q * 4 + 4, :], vws[:])

        # ================= morphology =================
        with tc.tile_pool(name="pC", bufs=1) as pC, \
                tc.tile_pool(name="psC", bufs=2, space="PSUM") as psC:
            bandt = {}
            for n, dtile in band_dram.items():
                r, c_ = bands[n].shape
                bandt[n] = pC.tile([r, c_], BF16, tag=f"bm_{n}",
                                   name=f"bm_{n}")
                nc.sync.dma_start(bandt[n][:], dtile[:])

            def thresh(dst, psum_ap, thr):
                nc.vector.tensor_single_scalar(dst, psum_ap, thr, ALU.is_gt)

            def padded(src_ap, rows, cols, pad, name):
                t = pC.tile([rows, cols + 2 * pad], BF16, tag=name)
                nc.vector.memset(t[:, 0:pad], 0.0)
                nc.vector.memset(t[:, pad + cols:], 0.0)
                nc.vector.tensor_copy(t[:, pad:pad + cols], src_ap)
                return t

            def se_conv2(src_list, band_prefix, wmap, out_psum, ncols, pad):
                groups = sorted(cls_groups(wmap).items())
                mms = []
                for tl, suff in src_list:
                    for wdt, _dys in groups:
                        hwt = pC.tile([tl.shape[0], ncols], BF16, name="hwt",
                                      tag=f"hw{band_prefix}{suff}{wdt}")
                        half = wdt // 2
                        nc.vector.tensor_copy(
                            hwt[:], tl[:, pad - half:pad - half + ncols])
                        for dd in range(1, wdt):
                            nc.vector.tensor_tensor(
                                hwt[:], hwt[:],
                                tl[:, pad - half + dd:pad - half + dd + ncols],
                                ALU.add)
                        mms.append((f"{band_prefix}{suff}_w{wdt}", hwt))
                for i, (bname, hwt) in enumerate(mms):
                    nc.tensor.matmul(out_psum[:], bandt[bname][:], hwt[:],
                                     start=(i == 0), stop=(i == len(mms) - 1))

            mp0 = padded(m0[:], 128, W, 3, "mp0")
            ps1 = psC.tile([128, W], F32, tag="psm")
            se_conv2([(mp0, "")], "d2", d2w, ps1, W, 3)
            m1t = pC.tile([128, W], BF16, tag="m1t")
            thresh(m1t[:], ps1[:], 12.5)
            mp1 = padded(m1t[:], 128, W, 3, "mp1")
            ps2 = psC.tile([128, W], F32, tag="psm")
            se_conv2([(mp1, "")], "d2", d2w, ps2, W, 3)
            m2t = pC.tile([128, W], BF16, tag="m2t")
            thresh(m2t[:], ps2[:], 0.5)
            mp2 = padded(m2t[:], 128, W, 3, "mp2")
            ps3 = psC.tile([128, W], F32, tag="psm")
            se_conv2([(mp2, "")], "d1", d1w, ps3, W, 3)
            m3t = pC.tile([128, W], BF16, tag="m3t")
            thresh(m3t[:], ps3[:], 0.5)
            mp3 = padded(m3t[:], 128, W, 3, "mp3")
            ps4 = psC.tile([128, W], F32, tag="psm")
            se_conv2([(mp3, "")], "d1", d1w, ps4, W, 3)
            m4t = pC.tile([128, W], BF16, tag="m4t")
            thresh(m4t[:], ps4[:], 4.5)
            mp4 = padded(m4t[:], 128, W, 6, "mp4")
            NC3 = 134

            def se_conv3(src_pad_tile, prefix, wmap, ncols, center_off):
                groups = sorted(cls_groups(wmap).items())
                mms = []
                for wdt, _dys in groups:
                    hwt = pC.tile([src_pad_tile.shape[0], ncols], BF16,
                                  name="hwt", tag=f"hw{prefix}{wdt}")
                    half = wdt // 2
                    base = center_off - half
                    nc.vector.tensor_copy(hwt[:],
                                          src_pad_tile[:, base:base + ncols])
                    for dd in range(1, wdt):
                        nc.vector.tensor_tensor(
                            hwt[:], hwt[:],
                            src_pad_tile[:, base + dd:base + dd + ncols],
                            ALU.add)
                    mms.append((wdt, hwt))
                return mms

            psda = psC.tile([67, NC3], F32, tag="psd3")
            mms = se_conv3(mp4, "d3", d3w, NC3, 3)
            for i, (wdt, hwt) in enumerate(mms):
                nc.tensor.matmul(psda[:], bandt[f"d3a_w{wdt}"][:], hwt[:],
                                 start=(i == 0), stop=(i == len(mms) - 1))
            Da = pC.tile([67, NC3], BF16, tag="Da")
            thresh(Da[:], psda[:], 0.5)
            psdb = psC.tile([67, NC3], F32, tag="psd3")
            for i, (wdt, hwt) in enumerate(mms):
                nc.tensor.matmul(psdb[:], bandt[f"d3b_w{wdt}"][:], hwt[:],
                                 start=(i == 0), stop=(i == len(mms) - 1))
            Db = pC.tile([67, NC3], BF16, tag="Db")
            thresh(Db[:], psdb[:], 0.5)
            pse = psC.tile([128, W], F32, tag="psm")
            mmsa = se_conv3(Da, "e3a", d3w, W, 3)
            mmsb = se_conv3(Db, "e3b", d3w, W, 3)
            allmm = [("e3a", wdt, hwt) for wdt, hwt in mmsa] + \
                    [("e3b", wdt, hwt) for wdt, hwt in mmsb]
            for i, (pref, wdt, hwt) in enumerate(allmm):
                nc.tensor.matmul(pse[:], bandt[f"{pref}_w{wdt}"][:], hwt[:],
                                 start=(i == 0), stop=(i == len(allmm) - 1))
            vfin = pC.tile([128, W], BF16, tag="vfin")
            # V = (erode_conv <= 28.5)
            nc.vector.tensor_single_scalar(vfin[:], pse[:], 28.5, ALU.is_le)
            nc.sync.dma_start(vfd[0:1, :], vfin[:])

        # ================= loop2 =================
        with tc.tile_pool(name="pD", bufs=2) as pD, \
                tc.tile_pool(name="pD1", bufs=2) as pD1, \
                tc.tile_pool(name="psD", bufs=2, space="PSUM") as psD, \
                tc.tile_pool(name="psDs", bufs=2, space="PSUM") as psDs, \
                tc.tile_pool(name="psDz", bufs=2, space="PSUM") as psDz, \
                tc.tile_pool(name="psDo", bufs=2, space="PSUM") as psDo:
            for s in range(NS):
                px0 = s * PX
                bp2 = pD.tile([128, 2, PX], BF16, tag="bp2")
                nc.sync.dma_start(bp2[:], bpd[:, :, px0:px0 + PX])
                xps = pD.tile([128, 2, PX], BF16, tag="xps")
                nc.sync.dma_start(xps[:], xp[:, :, px0:px0 + PX])
                vfs = pD.tile([1, PX], BF16, tag="vfs")
                nc.sync.dma_start(vfs[:], vfd[:, px0:px0 + PX])

                # R = cg*(K^T bp + u)
                Rt = pD1.tile([128, 2, PX], BF16, tag="Rt")
                for go in range(2):
                    for kb in range(PX // 512):
                        sl = slice(kb * 512, (kb + 1) * 512)
                        pp = psD.tile([128, 512], F32, tag="pconvD")
                        for gi in range(2):
                            nc.tensor.matmul(pp[:], wt["wr"][:, gi, go, :],
                                             bp2[:, gi, sl],
                                             start=(gi == 0), stop=(gi == 1))
                        nc.scalar.activation(Rt[:, go, sl], pp[:],
                                             AF.Identity,
                                             bias=cgu[:, go:go + 1],
                                             scale=cg[:, go:go + 1])

                osb = pD1.tile([128, 2, PX], F32, tag="osb")
                for hp in range(SR // 2):
                    o = hp * 2 * W
                    pl = psDs.tile([128, 2, 128], F32, tag="plD")
                    for hh in range(2):
                        oo = o + hh * W
                        for gi in range(2):
                            nc.tensor.matmul(pl[:, hh, :],
                                             Rt[:, gi, oo:oo + W],
                                             XC[:, gi, px0 + oo:
                                                px0 + oo + W],
                                             start=(gi == 0), stop=(gi == 1))
                    E = pD.tile([128, 2, 128], BF16, tag="ED")
                    nc.scalar.activation(E[:], pl[:], AF.Exp)
                    rs = pD.tile([128, 2], F32, tag="rsD")
                    nc.vector.tensor_reduce(rs[:], E[:], AX.X, ALU.add)
                    rr = pD.tile([128, 2], F32, tag="rrD")
                    nc.vector.reciprocal(rr[:], rs[:])
                    En = pD.tile([128, 2, 128], BF16, tag="EnD")
                    for hh in range(2):
                        nc.vector.tensor_scalar_mul(En[:, hh, :], E[:, hh, :],
                                                    rr[:, hh:hh + 1])
                    # reuse the (dead after exp) scores psum bytes for the
                    # transpose target to stay within the 8 PSUM banks
                    pm = pl[:].bitcast(BF16)[:, :, 0:128]
                    for hh in range(2):
                        nc.tensor.transpose(pm[:, hh, :], En[:, hh, :],
                                            identb[:])
                    m1tt = pD.tile([128, 2, 128], BF16, tag="m1tt")
                    nc.scalar.copy(m1tt[:], pm)
                    pz = psDz.tile([128, 2, 256], F32, tag="pzD")
                    for hh in range(2):
                        oo = px0 + o + hh * W
                        for gi in range(2):
                            nc.tensor.matmul(pz[:, hh, :],
                                             XC[:, gi, oo:oo + W],
                                             wt["gt"][:, gi, :, :].rearrange(
                                                 "p a b -> p (a b)"),
                                             start=(gi == 0), stop=(gi == 1))
                    zts = pD.tile([128, 2, 256], BF16, tag="zts")
                    nc.vector.tensor_copy(zts[:], pz[:])
                    po = psDo.tile([128, 2, 2, 128], F32, tag="poD")
                    for g2 in range(2):
                        pog = po[:, g2, :, :].rearrange("p b w -> p (b w)")
                        for gi in range(2):
                            nc.tensor.matmul(pog, wt["f2t"][:, gi, g2, :],
                                             xps[:, gi, o:o + 2 * W],
                                             start=(gi == 0), stop=False)
                        for hh in range(2):
                            nc.tensor.matmul(po[:, g2, hh, :],
                                             zts[:, hh,
                                                 g2 * 128:(g2 + 1) * 128],
                                             m1tt[:, hh, :],
                                             start=False, stop=False)
                        nc.tensor.matmul(pog, t_fwvrow[:, g2, :],
                                         vfs[:, o:o + 2 * W],
                                         start=False, stop=True)
                        nc.scalar.activation(
                            osb[:, g2, o:o + 2 * W], pog, AF.Identity,
                            bias=bt["bi_cb"][:, g2:g2 + 1])
                nc.sync.dma_start(y[:, :, px0:px0 + PX], osb[:])

    return nc


# ======================= top-level entry =======================
_CACHE = {}


def _get_nc():
    if "nc" not in _CACHE:
        nc = bass.Bass("TRN2", num_devices=8)
        build(nc)
        _CACHE["nc"] = nc
    return _CACHE["nc"]


def kernel(**inputs):
    nc = _get_nc()
    shared = prep_shared(inputs)
    x_p = np.asarray(inputs["x_p"], dtype=np.float32)
    x_c = np.asarray(inputs["x_c"], dtype=np.float32)
    in_maps = []
    for b in range(8):
        m = dict(shared)
        m["xp"] = prep_image(x_p[b])
        m["xc"] = prep_image(x_c[b])
        in_maps.append(m)
    res = run_bass_kernel_spmd(nc, in_maps, core_ids=list(range(8)))
    out = np.stack([post_image(r["y"]) for r in res.results])
    return np.ascontiguousarray(out, dtype=np.float32)
